# revision 1
# baseline (speedup 1.0000x reference)
"""Trainium2 Bass kernel for nn_CrossAttn_5763846111589 (retrieval_knn).

Pipeline per 128-query tile (data-parallel over N across 8 cores):
  1. PE: neighbor key matrix key[q,r] = 2*q.r - |r|^2  (argmax_8 == 8-NN)
  2. DVE: max / max_index -> top-8 values + ref indices
  3. GPSIMD indirect DMA: gather k_feat / v_feat rows for the 8 neighbors
  4. DVE/ACT: tiny softmax cross-attention over K=8
  5. PE: folded 1x1 conv  out = pred @ (W_out W_o W_v).T + bc
     (bias/weight folding is exact up to fp32 rounding; computed on host in fp64)
"""

import sys

sys.path.insert(0, "/opt/trn_rl_repo")

import numpy as np

import concourse.bass as bass
import concourse.mybir as mybir
import concourse.tile as tile
from concourse.masks import make_identity

F32 = mybir.dt.float32
BF16 = mybir.dt.bfloat16
U32 = mybir.dt.uint32
CONTR = 4  # key-matmul contraction rows: [2x, 2y, 2z, -1] (fp32)

N = 32768
M = 8192
C = 128
K = 8
N_CORES = 8
N_CORE = N // N_CORES  # 4096 queries per core
P = 128  # queries per tile (partition dim)
RB = 512  # refs per key-matmul block (one PSUM bank of fp32)

_WSPLIT_CTR = [0]


def split_waits(nc, limit=1):
    """The pinned walrus encodes only ONE sync wait per instruction; split
    extra waits into single-wait NoOps on the same engine right before the
    instruction (the sequencer executes waits in stream order, so this is
    semantically identical)."""
    n_split = 0
    for fn in nc.m.functions:
        for blk in fn.blocks:
            new_list = []
            for ins in blk.instructions:
                si = ins.sync_info
                if si is not None and len(si.on_wait) > limit:
                    waits = list(si.on_wait)
                    for w in waits[:-limit]:
                        _WSPLIT_CTR[0] += 1
                        nop = mybir.InstNoOp(
                            name=f"WSPLIT-{_WSPLIT_CTR[0]}", ins=[], outs=[]
                        )
                        nop.engine = ins.engine
                        nop.sync_info = mybir.SyncInfo(on_wait=[w], on_update=[])
                        new_list.append(nop)
                    ins.sync_info = mybir.SyncInfo(
                        on_wait=waits[-limit:], on_update=list(si.on_update)
                    )
                    n_split += 1
                new_list.append(ins)
            blk.instructions = new_list
    return n_split


def build_program(n_core=N_CORE, m=M, c=C, k=K, rb=RB, split=True, repeat=1):
    """Build the per-core Bass program (SPMD: same program on all cores)."""
    nc = bass.Bass("TRN2", debug=False, target_bir_lowering=False)

    qT_d = nc.dram_tensor("qT", [CONTR, n_core], F32, kind="ExternalInput")
    refT_d = nc.dram_tensor("refT", [CONTR, m], F32, kind="ExternalInput")
    qf_d = nc.dram_tensor("q_feat", [n_core, c], F32, kind="ExternalInput")
    kf_d = nc.dram_tensor("k_feat", [m, c], F32, kind="ExternalInput")
    vf_d = nc.dram_tensor("v_feat", [m, c], F32, kind="ExternalInput")
    WcT_d = nc.dram_tensor("WcT", [c, c], F32, kind="ExternalInput")
    bc_d = nc.dram_tensor("bc_bcast", [P, c], F32, kind="ExternalInput")
    out_d = nc.dram_tensor("out", [n_core, c], F32, kind="ExternalOutput")

    n_tiles = n_core // P
    n_blocks = m // rb
    inv_sqrt_c = 1.0 / float(np.sqrt(c))

    with tile.TileContext(nc) as tc:
        with (
            tc.tile_pool(name="const", bufs=1) as const,
            tc.tile_pool(name="keyrow", bufs=2) as keyrow,
            tc.tile_pool(name="pk", bufs=2, space="PSUM") as pk_pool,
            tc.tile_pool(name="pmm", bufs=2, space="PSUM") as pmm_pool,
            tc.tile_pool(name="small", bufs=3) as small,
            tc.tile_pool(name="gath", bufs=2) as gath,
            tc.tile_pool(name="ot", bufs=3) as ot,
        ):
            qT = const.tile([CONTR, n_core], F32)
            refT = const.tile([CONTR, m], F32)
            WcT = const.tile([c, c], F32)
            bc = const.tile([P, c], F32)
            ident = const.tile([P, P], F32)
            # Matmult/Ldweights codegen allows only ONE sync wait, so every
            # tensor a PE instruction reads must have a single-engine writer:
            # key-matmul inputs staged via ACT (same sem as the PSUM-release
            # copies), transpose/out-matmul inputs staged via DVE.
            qT_ld = const.tile([CONTR, n_core], F32)
            refT_ld = const.tile([CONTR, m], F32)
            WcT_ld = const.tile([c, c], F32)
            ident_ld = const.tile([P, P], F32)
            nc.sync.dma_start(qT_ld[:], qT_d[:])
            nc.sync.dma_start(refT_ld[:], refT_d[:])
            nc.sync.dma_start(WcT_ld[:], WcT_d[:])
            nc.sync.dma_start(bc[:], bc_d[:])
            make_identity(nc, ident_ld[:])
            nc.scalar.copy(qT[:], qT_ld[:])
            nc.scalar.copy(refT[:], refT_ld[:])
            nc.vector.tensor_copy(WcT[:], WcT_ld[:])
            nc.vector.tensor_copy(ident[:], ident_ld[:])

            for t in list(range(n_tiles)) * repeat:
                qf = ot.tile([P, c], F32, tag="qf")
                nc.sync.dma_start(qf[:], qf_d[t * P : (t + 1) * P, :])

                # --- 1. key matrix: key[q, r] = 2 q.r - |r|^2 ---
                key = keyrow.tile([P, m], F32)
                # Claim the key slot with one tiny ACT write: it alone carries
                # the DVE slot-release wait, keeping every per-block PSUM->SBUF
                # copy at <=2 sync waits (ACT codegen limit).
                nc.scalar.mul(key[:, 0:1], ident[:, 0:1], 0.0)
                for b in range(n_blocks // 2):
                    # two matmuls fill a 2-bank PSUM tile; one wide ACT copy
                    pk = pk_pool.tile([P, 2 * rb], F32)
                    for h in range(2):
                        nc.tensor.matmul(
                            pk[:, h * rb : (h + 1) * rb],
                            lhsT=qT[:, t * P : (t + 1) * P],
                            rhs=refT[:, (2 * b + h) * rb : (2 * b + h + 1) * rb],
                            start=True,
                            stop=True,
                        )
                    nc.scalar.copy(
                        key[:, 2 * b * rb : 2 * (b + 1) * rb], pk[:]
                    )

                # --- 2. top-8 (largest key == nearest) ---
                vals = small.tile([P, 8], F32, tag="vals")
                idx = small.tile([P, 8], U32, tag="idx")
                nc.vector.max(out=vals[:], in_=key[:])
                nc.vector.max_index(out=idx[:], in_max=vals[:], in_values=key[:])

                # --- 3. gather neighbor features (rows of k_feat / v_feat) ---
                # HW generates one descriptor per partition per indirect DMA
                # (consuming a single offset), so gather the K neighbor rows
                # with K separate single-index DMAs.
                k_g = gath.tile([P, k * c], F32, tag="k_g")
                v_g = gath.tile([P, k * c], F32, tag="v_g")
                for j in range(k):
                    nc.gpsimd.indirect_dma_start(
                        out=k_g[:, j * c : (j + 1) * c],
                        out_offset=None,
                        in_=kf_d[:],
                        in_offset=bass.IndirectOffsetOnAxis(
                            ap=idx[:, j : j + 1], axis=0
                        ),
                    )
                    nc.gpsimd.indirect_dma_start(
                        out=v_g[:, j * c : (j + 1) * c],
                        out_offset=None,
                        in_=vf_d[:],
                        in_offset=bass.IndirectOffsetOnAxis(
                            ap=idx[:, j : j + 1], axis=0
                        ),
                    )

                # --- 4. attention: scores = (q . k_g)/sqrt(C); softmax; pred ---
                # multiply on Pool (frees DVE), grouped-reduce on DVE
                prod = gath.tile([P, k * c], F32, tag="prod")
                nc.gpsimd.tensor_tensor(
                    out=prod[:].rearrange("p (k c) -> p k c", k=k),
                    in0=k_g[:].rearrange("p (k c) -> p k c", k=k),
                    in1=qf[:, None, :].to_broadcast([P, k, c]),
                    op=mybir.AluOpType.mult,
                )
                raw = small.tile([P, k], F32, tag="raw")
                nc.vector.tensor_reduce(
                    out=raw[:],
                    in_=prod[:].rearrange("p (k c) -> p k c", k=k),
                    axis=mybir.AxisListType.X,
                    op=mybir.AluOpType.add,
                )
                rmax = small.tile([P, 1], F32, tag="rmax")
                nc.vector.tensor_reduce(
                    out=rmax[:],
                    in_=raw[:],
                    axis=mybir.AxisListType.X,
                    op=mybir.AluOpType.max,
                )
                nbias = small.tile([P, 1], F32, tag="nbias")
                nc.scalar.mul(nbias[:], rmax[:], -inv_sqrt_c)
                exp_s = small.tile([P, k], F32, tag="exp_s")
                sumexp = small.tile([P, 1], F32, tag="sumexp")
                nc.scalar.activation(
                    exp_s[:],
                    raw[:],
                    mybir.ActivationFunctionType.Exp,
                    bias=nbias[:],
                    scale=inv_sqrt_c,
                    accum_out=sumexp[:],
                )
                recip = small.tile([P, 1], F32, tag="recip")
                nc.vector.reciprocal(recip[:], sumexp[:])
                attn = small.tile([P, k], F32, tag="attn")
                nc.vector.tensor_scalar(
                    attn[:], exp_s[:], recip[:], None, op0=mybir.AluOpType.mult
                )

                # pred = sum_j attn_j * v_j (fused multiply-accumulate chain)
                pred = ot.tile([P, c], F32, tag="pred")
                nc.vector.tensor_scalar(
                    pred[:], v_g[:, 0:c], attn[:, 0:1], None,
                    op0=mybir.AluOpType.mult,
                )
                for j in range(1, k):
                    nc.vector.scalar_tensor_tensor(
                        out=pred[:],
                        in0=v_g[:, j * c : (j + 1) * c],
                        scalar=attn[:, j : j + 1],
                        in1=pred[:],
                        op0=mybir.AluOpType.mult,
                        op1=mybir.AluOpType.add,
                    )

                # --- 5. folded 1x1 convs: out = pred @ Wc.T + bc ---
                predT_ps = pmm_pool.tile([P, P], F32, tag="predT_ps")
                nc.tensor.transpose(predT_ps[:], pred[:], ident[:])
                predT = ot.tile([P, P], F32, tag="predT")
                # DVE (not ACT) so the following matmul's deps (this copy +
                # o_ps release by the DVE bias-add) collapse to one semaphore.
                nc.vector.tensor_copy(predT[:], predT_ps[:])
                o_ps = pmm_pool.tile([P, c], F32, tag="o_ps")
                nc.tensor.matmul(
                    o_ps[:], lhsT=predT[:], rhs=WcT[:], start=True, stop=True
                )
                o_sb = ot.tile([P, c], F32, tag="o_sb")
                nc.vector.tensor_tensor(
                    out=o_sb[:], in0=o_ps[:], in1=bc[:], op=mybir.AluOpType.add
                )
                nc.sync.dma_start(out_d[t * P : (t + 1) * P, :], o_sb[:])

    if split:
        split_waits(nc)
    return nc


def _bf16_split3(x):
    """x (fp32) -> (h, m, l) bf16 with h+m+l capturing ~24 mantissa bits."""
    import ml_dtypes

    bf = ml_dtypes.bfloat16
    h = x.astype(bf)
    r = x - h.astype(np.float32)
    mm = r.astype(bf)
    l = (r - mm.astype(np.float32)).astype(bf)
    return h, mm, l


def build_qT(xyz_q):
    """[CONTR, n] fp32 lhs rows for key[q,r] = 2 q.r - |r|^2.

    fp32 PE matmul keeps the key's rounding close to the reference's own
    fp32 distance computation, minimizing near-tie neighbor disagreements.
    """
    n = xyz_q.shape[0]
    t = 2.0 * xyz_q.astype(np.float32)
    return np.ascontiguousarray(
        np.concatenate([t.T, -np.ones((1, n), np.float32)], axis=0)
    )


def build_refT(xyz_ref):
    """[CONTR, m] fp32 rhs rows [x, y, z, |r|^2]."""
    ref_sq = np.sum(xyz_ref.astype(np.float64) ** 2, axis=-1).astype(np.float32)
    return np.ascontiguousarray(
        np.concatenate(
            [xyz_ref.T.astype(np.float32), ref_sq[None, :]], axis=0
        )
    )


def prep_inputs(xyz_pred, xyz_ref, q_feat, k_feat, v_feat, W_v, b_v, W_o, b_o, W_out, b_out):
    """Host-side layout prep. Returns per-core in_maps."""
    Wc = (
        W_out.astype(np.float64) @ W_o.astype(np.float64) @ W_v.astype(np.float64)
    )
    bc = (
        W_out.astype(np.float64) @ W_o.astype(np.float64) @ b_v.astype(np.float64)
        + W_out.astype(np.float64) @ b_o.astype(np.float64)
        + b_out.astype(np.float64)
    )
    WcT = np.ascontiguousarray(Wc.T.astype(np.float32))
    bc_bcast = np.ascontiguousarray(
        np.broadcast_to(bc.astype(np.float32)[None, :], (P, C))
    )

    refT = build_refT(xyz_ref)

    k_feat = np.ascontiguousarray(k_feat.astype(np.float32))
    v_feat = np.ascontiguousarray(v_feat.astype(np.float32))

    in_maps = []
    for core in range(N_CORES):
        sl = slice(core * N_CORE, (core + 1) * N_CORE)
        qT = build_qT(xyz_pred[sl].astype(np.float32))
        in_maps.append(
            {
                "qT": np.ascontiguousarray(qT),
                "refT": refT,
                "q_feat": np.ascontiguousarray(q_feat[sl].astype(np.float32)),
                "k_feat": k_feat,
                "v_feat": v_feat,
                "WcT": WcT,
                "bc_bcast": bc_bcast,
            }
        )
    return in_maps


TRACE = False
LAST_RESULTS = None


def kernel(**inputs):
    global LAST_RESULTS
    from concourse.bass_utils import run_bass_kernel_spmd

    in_maps = prep_inputs(**{k: np.asarray(v) for k, v in inputs.items()})
    nc = build_program()
    res = run_bass_kernel_spmd(
        nc, in_maps, core_ids=list(range(N_CORES)), trace=TRACE
    )
    LAST_RESULTS = res
    out = np.concatenate([r["out"] for r in res.results], axis=0)
    return out.astype(np.float32)


if __name__ == "__main__":
    rng = np.random.default_rng(0)
    ins = {
        "xyz_pred": rng.normal(size=(N, 3)).astype(np.float32) * 10,
        "xyz_ref": rng.normal(size=(M, 3)).astype(np.float32) * 10,
        "q_feat": rng.normal(size=(N, C)).astype(np.float32),
        "k_feat": rng.normal(size=(M, C)).astype(np.float32),
        "v_feat": rng.normal(size=(M, C)).astype(np.float32),
        "W_v": rng.normal(size=(C, C)).astype(np.float32),
        "b_v": rng.normal(size=(C,)).astype(np.float32),
        "W_o": rng.normal(size=(C, C)).astype(np.float32),
        "b_o": rng.normal(size=(C,)).astype(np.float32),
        "W_out": rng.normal(size=(C, C)).astype(np.float32),
        "b_out": rng.normal(size=(C,)).astype(np.float32),
    }
    out = kernel(**ins)
    print(out.shape, out.dtype)

